# revision 3
# baseline (speedup 1.0000x reference)
"""Additive (Bahdanau) attention on 8 TRN2 NeuronCores.

Reference computation:
    qp = queries @ W_q                  (bs, n_q, 64)
    kp = keys @ W_k                     (bs, n_k, 64)
    scores[b,q,k] = sum_h w_v[h] * tanh(qp[b,q,h] + kp[b,k,h])
    out = softmax(scores, -1) @ values

Key trick: tanh(x) on [-9.9, 9.9] is approximated by a sum of J=8 sines
(non-harmonic frequencies, max err 1.4e-3):
    tanh(x) ~= sum_j c_j sin(w_j x)
Angle addition makes the score computation separable:
    sin(w_j(a+b)) = sin(w_j a)cos(w_j b) + cos(w_j a)sin(w_j b)
so scores = [SQ|CQ] contracted against [CK|SK] over 128 partitions per j —
pure TensorEngine work. The giant (bs, n_q, n_k, 64) tanh tensor of the
naive implementation never exists.

sin/cos args reach |w_j x| ~ 21 rad but the ScalarE Sin spline only covers
[-4, 4], so angles are range-reduced exactly in fp32:
    z = x * (w_j / 2pi) + (S + 32)      # S = 0 (sin half) / 0.25 (cos half)
    r = (z + 2^23) - 2^23               # round-to-nearest via fp32 magic
    g = r - z                           # in [-0.5, 0.5], g = -frac
    sin(w_j x + 2pi S) = Sin(-2pi * g)  # exact periodicity

Sharding: fully data-parallel, no collectives. Core c handles batch c//2,
query half c%2: (512 q, 1024 k).
"""

import numpy as np

BS, NQ, NK = 4, 1024, 1024
QD, KD, VD, HID = 128, 128, 128, 64
NCORES = 8
NQH = NQ // 2  # queries per core

J = 8
FOURIER_W = [
    0.2667839792044199, 1.9094257132362642, 0.8040518809611484,
    2.4802060085246405, 1.3507588730127413, 3.061945054980018,
    3.651547203952487, 4.2343878787163245,
]
FOURIER_C = [
    1.2382308242762494, 0.056420304494510194, 0.33277866141571627,
    0.023446006114235703, 0.13433221835158662, 0.009559675468884234,
    0.003811486064186856, 0.0014226822306590728,
]

MAGIC = 8388608.0  # 2^23
TWO_PI = 6.283185307179586

_CACHED = {}


def _build():
    import concourse.bacc as bacc
    import concourse.mybir as mybir
    from concourse import tile
    from concourse.alu_op_type import AluOpType
    from concourse.masks import make_identity

    F32 = mybir.dt.float32
    BF16 = mybir.dt.bfloat16
    A = mybir.ActivationFunctionType

    nc = bacc.Bacc(None, target_bir_lowering=False)

    q_sh = nc.declare_dram_parameter("q_sh", [NQH, QD], F32, isOutput=False)
    k_sh = nc.declare_dram_parameter("k_sh", [NK, KD], F32, isOutput=False)
    v_sh = nc.declare_dram_parameter("v_sh", [NK, VD], F32, isOutput=False)
    wq2 = nc.declare_dram_parameter("wq2", [QD, 128], F32, isOutput=False)
    wk2 = nc.declare_dram_parameter("wk2", [KD, 128], F32, isOutput=False)
    cw = nc.declare_dram_parameter("cw", [128, J], F32, isOutput=False)
    sphq = nc.declare_dram_parameter("sphq", [128, 1], F32, isOutput=False)
    sphk = nc.declare_dram_parameter("sphk", [128, 1], F32, isOutput=False)
    out = nc.declare_dram_parameter("out", [NQH, VD], F32, isOutput=True)

    NQC = NQH // 128  # 4 query chunks
    NKC = NK // 128   # 8 key chunks

    with tile.TileContext(nc) as tc:
        with (
            tc.tile_pool(name="consts", bufs=1) as consts,
            tc.tile_pool(name="io", bufs=1) as io,
            tc.tile_pool(name="chunks", bufs=3) as chunks,
            tc.tile_pool(name="vals", bufs=NKC) as vals,
            tc.tile_pool(name="work", bufs=3) as work,
            tc.tile_pool(name="jbank", bufs=J) as jbank,
            tc.tile_pool(name="sm", bufs=8) as sm,
            tc.tile_pool(name="attn32", bufs=4 * NKC) as attn32,
            tc.tile_pool(name="ps512", bufs=4, space="PSUM") as ps512,
            tc.tile_pool(name="ps128", bufs=2, space="PSUM") as ps128,
            tc.tile_pool(name="psb16", bufs=2, space="PSUM") as psb16,
        ):
            # ---- constants ----
            id32 = consts.tile([128, 128], F32, tag="id32")
            id16 = consts.tile([128, 128], BF16, tag="id16")
            make_identity(nc, id32[:])
            make_identity(nc, id16[:])
            wq2_sb = consts.tile([QD, 128], F32, tag="wq2")
            wk2_sb = consts.tile([KD, 128], F32, tag="wk2")
            cw_sb = consts.tile([128, J], F32, tag="cw")
            sphq_sb = consts.tile([128, 1], F32, tag="sphq")
            sphk_sb = consts.tile([128, 1], F32, tag="sphk")
            nc.sync.dma_start(wq2_sb[:], wq2[:])
            nc.sync.dma_start(wk2_sb[:], wk2[:])
            nc.sync.dma_start(cw_sb[:], cw[:])
            nc.sync.dma_start(sphq_sb[:], sphq[:])
            nc.sync.dma_start(sphk_sb[:], sphk[:])

            # ---- inputs: DMA per 128-row chunk, transpose q/k, cast v ----
            qT = io.tile([QD, NQH], F32, tag="qT")
            kT = io.tile([KD, NK], F32, tag="kT")
            v16 = [vals.tile([128, VD], BF16, tag="v16", name=f"v16_{i}")
                   for i in range(NKC)]
            for i in range(NQC):
                qc = chunks.tile([128, QD], F32, tag="qc")
                nc.sync.dma_start(qc[:], q_sh[i * 128:(i + 1) * 128, :])
                p = ps128.tile([128, 128], F32, tag="t128")
                nc.tensor.transpose(p[:], qc[:], id32[:])
                nc.scalar.copy(qT[:, i * 128:(i + 1) * 128], p[:])
            for i in range(NKC):
                kc_t = chunks.tile([128, KD], F32, tag="kc")
                nc.sync.dma_start(kc_t[:], k_sh[i * 128:(i + 1) * 128, :])
                p = ps128.tile([128, 128], F32, tag="t128")
                nc.tensor.transpose(p[:], kc_t[:], id32[:])
                nc.scalar.copy(kT[:, i * 128:(i + 1) * 128], p[:])
                vc = chunks.tile([128, VD], F32, tag="vc")
                nc.sync.dma_start(vc[:], v_sh[i * 128:(i + 1) * 128, :])
                nc.vector.tensor_copy(v16[i][:], vc[:])

            # ---- projections: packed (2x64 h, n) = [W | W]^T @ xT ----
            qp2 = io.tile([128, NQH], F32, tag="qp2")
            kp2 = io.tile([128, NK], F32, tag="kp2")
            p = ps512.tile([128, NQH], F32, tag="t512")
            nc.tensor.matmul(p[:], wq2_sb[:], qT[:], start=True, stop=True)
            nc.scalar.copy(qp2[:], p[:])
            for c in range(2):
                p = ps512.tile([128, 512], F32, tag="t512")
                nc.tensor.matmul(p[:], wk2_sb[:], kT[:, c * 512:(c + 1) * 512],
                                 start=True, stop=True)
                nc.scalar.copy(kp2[:, c * 512:(c + 1) * 512], p[:])

            # ---- per-j sin/cos banks via exact range reduction ----
            # Q side rows: [sin | cos] (phases 32.0 | 32.25), scaled by c_j*w_v
            # K side rows: [cos | sin] (phases 32.25 | 32.0) so one 128-deep
            # contraction computes sin_q*cos_k + cos_q*sin_k.
            KS = []
            SQ = []
            for j in range(J):
                s1 = float(FOURIER_W[j] / TWO_PI)
                zk = work.tile([128, NK], F32, tag="zk")
                rk = work.tile([128, NK], F32, tag="rk")
                gk = work.tile([128, NK], F32, tag="gk")
                ks = jbank.tile([128, NK], BF16, tag="ks")
                nc.vector.tensor_scalar(zk[:], kp2[:], s1, sphk_sb[:],
                                        AluOpType.mult, AluOpType.add)
                nc.vector.tensor_scalar(rk[:], zk[:], MAGIC, MAGIC,
                                        AluOpType.add, AluOpType.subtract)
                nc.gpsimd.tensor_sub(gk[:], rk[:], zk[:])
                nc.scalar.activation(ks[:], gk[:], A.Sin, scale=-TWO_PI)
                KS.append(ks)

                zq = work.tile([128, NQH], F32, tag="zq")
                rq = work.tile([128, NQH], F32, tag="rq")
                gq = work.tile([128, NQH], F32, tag="gq")
                sq_f = work.tile([128, NQH], F32, tag="sqf")
                sq = jbank.tile([128, NQH], BF16, tag="sq")
                nc.vector.tensor_scalar(zq[:], qp2[:], s1, sphq_sb[:],
                                        AluOpType.mult, AluOpType.add)
                nc.vector.tensor_scalar(rq[:], zq[:], MAGIC, MAGIC,
                                        AluOpType.add, AluOpType.subtract)
                nc.gpsimd.tensor_sub(gq[:], rq[:], zq[:])
                nc.scalar.activation(sq_f[:], gq[:], A.Sin, scale=-TWO_PI)
                nc.vector.tensor_scalar_mul(sq[:], sq_f[:], cw_sb[:, j:j + 1])
                SQ.append(sq)

            # ---- scores (q-major tiles) + exp with row-sum accumulation ----
            expt = [[None] * 2 for _ in range(NQC)]
            sump = [[None] * 2 for _ in range(NQC)]
            for qt in range(NQC):
                for kc in range(2):
                    psc = ps512.tile([128, 512], F32, tag="t512")
                    for j in range(J):
                        nc.tensor.matmul(
                            psc[:], SQ[j][:, qt * 128:(qt + 1) * 128],
                            KS[j][:, kc * 512:(kc + 1) * 512],
                            start=(j == 0), stop=(j == J - 1))
                    et = sm.tile([128, 512], BF16, tag="expt")
                    smp = sm.tile([128, 1], F32, tag="sump")
                    nc.scalar.activation(et[:], psc[:], A.Exp, accum_out=smp[:])
                    expt[qt][kc] = et
                    sump[qt][kc] = smp

            # ---- softmax denominators ----
            rsum = []
            for qt in range(NQC):
                ss = sm.tile([128, 1], F32, tag="ssum")
                rs = sm.tile([128, 1], F32, tag="rsum")
                nc.vector.tensor_add(ss[:], sump[qt][0][:], sump[qt][1][:])
                nc.vector.reciprocal(rs[:], ss[:])
                rsum.append(rs)

            # ---- transpose attention, weighted sum of values, normalize ----
            attnT = [[None] * NQC for _ in range(NKC)]
            for qt in range(NQC):
                for kc in range(2):
                    for c in range(4):
                        kchunk = kc * 4 + c
                        pt = psb16.tile([128, 128], BF16, tag="tb128")
                        at = attn32.tile([128, 128], BF16, tag="attnT")
                        nc.tensor.transpose(
                            pt[:], expt[qt][kc][:, c * 128:(c + 1) * 128], id16[:])
                        nc.vector.tensor_copy(at[:], pt[:])
                        attnT[kchunk][qt] = at

            for qt in range(NQC):
                po = ps128.tile([128, VD], F32, tag="t128")
                for kchunk in range(NKC):
                    nc.tensor.matmul(po[:], attnT[kchunk][qt][:], v16[kchunk][:],
                                     start=(kchunk == 0), stop=(kchunk == NKC - 1))
                o_sb = sm.tile([128, VD], F32, tag="osb")
                nc.vector.tensor_scalar_mul(o_sb[:], po[:], rsum[qt][:])
                nc.sync.dma_start(out[qt * 128:(qt + 1) * 128, :], o_sb[:])

    nc.finalize()
    return nc


def _get_nc():
    if "nc" not in _CACHED:
        _CACHED["nc"] = _build()
    return _CACHED["nc"]


def _make_consts(W_q, W_k, w_v):
    wq2 = np.concatenate([W_q, W_q], axis=1).astype(np.float32)
    wk2 = np.concatenate([W_k, W_k], axis=1).astype(np.float32)
    cw = np.zeros((128, J), np.float32)
    for j in range(J):
        cwj = (FOURIER_C[j] * w_v).astype(np.float32)
        cw[:64, j] = cwj
        cw[64:, j] = cwj
    sphq = np.full((128, 1), 32.0, np.float32)
    sphq[64:] = 32.25
    sphk = np.full((128, 1), 32.25, np.float32)
    sphk[64:] = 32.0
    return wq2, wk2, cw, sphq, sphk


def kernel(queries, keys, values, W_q, W_k, w_v, _trace=False, _trace_kwargs=None):
    from concourse.bass_utils import run_bass_kernel_spmd

    nc = _get_nc()
    wq2, wk2, cw, sphq, sphk = _make_consts(
        np.asarray(W_q), np.asarray(W_k), np.asarray(w_v))
    queries = np.ascontiguousarray(queries, np.float32)
    keys = np.ascontiguousarray(keys, np.float32)
    values = np.ascontiguousarray(values, np.float32)

    in_maps = []
    for c in range(NCORES):
        b, qh = c // 2, c % 2
        in_maps.append({
            "q_sh": np.ascontiguousarray(queries[b, qh * NQH:(qh + 1) * NQH, :]),
            "k_sh": keys[b],
            "v_sh": values[b],
            "wq2": wq2, "wk2": wk2, "cw": cw, "sphq": sphq, "sphk": sphk,
        })

    kwargs = {}
    if _trace:
        kwargs["trace"] = True
        kwargs.update(_trace_kwargs or {})
    res = run_bass_kernel_spmd(nc, in_maps, core_ids=list(range(NCORES)), **kwargs)

    out = np.empty((BS, NQ, VD), np.float32)
    for c in range(NCORES):
        b, qh = c // 2, c % 2
        out[b, qh * NQH:(qh + 1) * NQH, :] = res.results[c]["out"]
    if _trace:
        return out, res
    return out


# revision 7
# speedup vs baseline: 1.1747x; 1.1747x over previous
"""Additive (Bahdanau) attention on 8 TRN2 NeuronCores.

Reference computation:
    qp = queries @ W_q                  (bs, n_q, 64)
    kp = keys @ W_k                     (bs, n_k, 64)
    scores[b,q,k] = sum_h w_v[h] * tanh(qp[b,q,h] + kp[b,k,h])
    out = softmax(scores, -1) @ values

Key trick: tanh(x) on [-9.9, 9.9] is approximated by a sum of J=8 sines
(non-harmonic frequencies, max err 1.4e-3):
    tanh(x) ~= sum_j c_j sin(w_j x)
Angle addition makes the score computation separable:
    sin(w_j(a+b)) = sin(w_j a)cos(w_j b) + cos(w_j a)sin(w_j b)
so scores reduce to matmuls with contraction dim J*2*64 — pure TensorEngine
work. The giant (bs, n_q, n_k, 64) tanh tensor of the naive implementation
never exists.

sin/cos args reach |w_j x| ~ 21 rad but the ScalarE Sin spline only covers
[-4, 4], so angles are range-reduced exactly in fp32:
    z = x * (w_j / 2pi) + (S + 32)      # S = 0 (sin half) / 0.25 (cos half)
    r = (z + 2^23) - 2^23               # round-to-nearest via fp32 magic
    g = r - z                           # in [-0.5, 0.5], g = -frac
    sin(w_j x + 2pi S) = Sin(-2pi * g)  # exact periodicity
(j=0 has small enough args to skip the reduction.)

Scores are built TRANSPOSED (k on partitions, q free) so the attention
weights feed the output matmul with no transposes:
    outT (v, q) = sum_kt values[kt]^T-contraction @ expT[kt]
    sums (1, q) = sum_kt ones^T @ expT[kt]
and only the final (v, q) -> (q, v) transpose + per-partition normalize
remain.

Sharding: fully data-parallel, no collectives. Core c handles batch c//2,
query half c%2: (512 q, 1024 k).
"""

import numpy as np

BS, NQ, NK = 4, 1024, 1024
QD, KD, VD, HID = 128, 128, 128, 64
NCORES = 8
NQH = NQ // 2  # queries per core

J = 8
FOURIER_W = [
    0.2667839792044199, 1.9094257132362642, 0.8040518809611484,
    2.4802060085246405, 1.3507588730127413, 3.061945054980018,
    3.651547203952487, 4.2343878787163245,
]
FOURIER_C = [
    1.2382308242762494, 0.056420304494510194, 0.33277866141571627,
    0.023446006114235703, 0.13433221835158662, 0.009559675468884234,
    0.003811486064186856, 0.0014226822306590728,
]
# process j in ascending-frequency order, smallest first (it skips the wrap)
J_ORDER = [0, 2, 4, 1, 3, 5, 6, 7]

MAGIC = 8388608.0  # 2^23
TWO_PI = 6.283185307179586
HALF_PI = 1.5707963267948966

_CACHED = {}


def _build():
    import concourse.bacc as bacc
    import concourse.mybir as mybir
    from concourse import tile
    from concourse.alu_op_type import AluOpType
    from concourse.masks import make_identity

    F32 = mybir.dt.float32
    BF16 = mybir.dt.bfloat16
    A = mybir.ActivationFunctionType

    nc = bacc.Bacc(None, target_bir_lowering=False)

    q_sh = nc.declare_dram_parameter("q_sh", [NQH, QD], F32, isOutput=False)
    k_sh = nc.declare_dram_parameter("k_sh", [NK, KD], F32, isOutput=False)
    v_sh = nc.declare_dram_parameter("v_sh", [NK, VD], F32, isOutput=False)
    wq2 = nc.declare_dram_parameter("wq2", [QD, 128], F32, isOutput=False)
    wk2 = nc.declare_dram_parameter("wk2", [KD, 128], F32, isOutput=False)
    cw = nc.declare_dram_parameter("cw", [128, J], F32, isOutput=False)
    sphq = nc.declare_dram_parameter("sphq", [128, 1], F32, isOutput=False)
    sphk = nc.declare_dram_parameter("sphk", [128, 1], F32, isOutput=False)
    # direct-path (j=0) activation bias: phase in radians [0|pi/2] (q), [pi/2|0] (k)
    biasq = nc.declare_dram_parameter("biasq", [128, 1], F32, isOutput=False)
    biask = nc.declare_dram_parameter("biask", [128, 1], F32, isOutput=False)
    out = nc.declare_dram_parameter("out", [NQH, VD], F32, isOutput=True)

    NQC = NQH // 128  # 4 query chunks
    NKC = NK // 128   # 8 key chunks

    with tile.TileContext(nc) as tc:
        with (
            tc.tile_pool(name="consts", bufs=1) as consts,
            tc.tile_pool(name="io", bufs=1) as io,
            tc.tile_pool(name="chunks", bufs=4) as chunks,
            tc.tile_pool(name="vals", bufs=NKC) as vals,
            tc.tile_pool(name="work", bufs=3) as work,
            tc.tile_pool(name="jbank", bufs=3) as jbank,
            tc.tile_pool(name="sm", bufs=NKC) as sm,
            tc.tile_pool(name="ps", bufs=8, space="PSUM") as ps,
        ):
            # ---- constants (sync queue; tiny) ----
            id32 = consts.tile([128, 128], F32, tag="id32")
            id16 = consts.tile([128, 128], BF16, tag="id16")
            make_identity(nc, id32[:])
            make_identity(nc, id16[:])
            ones16 = consts.tile([128, 1], BF16, tag="ones16")
            nc.gpsimd.memset(ones16[:], 1.0)
            wq2_sb = consts.tile([QD, 128], F32, tag="wq2")
            wk2_sb = consts.tile([KD, 128], F32, tag="wk2")
            cw_sb = consts.tile([128, J], F32, tag="cw")
            sphq_sb = consts.tile([128, 1], F32, tag="sphq")
            sphk_sb = consts.tile([128, 1], F32, tag="sphk")
            biasq_sb = consts.tile([128, 1], F32, tag="biasq")
            biask_sb = consts.tile([128, 1], F32, tag="biask")
            nc.sync.dma_start(wq2_sb[:], wq2[:])
            nc.sync.dma_start(wk2_sb[:], wk2[:])
            nc.sync.dma_start(cw_sb[:], cw[:])
            nc.sync.dma_start(sphq_sb[:], sphq[:])
            nc.sync.dma_start(sphk_sb[:], sphk[:])
            nc.sync.dma_start(biasq_sb[:], biasq[:])
            nc.sync.dma_start(biask_sb[:], biask[:])

            # ---- inputs: q/k spread across queues, transpose to (d, n) ----
            qT = io.tile([QD, NQH], F32, tag="qT")
            kT = io.tile([KD, NK], F32, tag="kT")
            for i in range(NQC):
                qc = chunks.tile([128, QD], F32, tag="qc")
                nc.sync.dma_start(qc[:], q_sh[i * 128:(i + 1) * 128, :])
                p = ps.tile([128, 512], F32, tag="t512")
                nc.tensor.transpose(p[:, :128], qc[:], id32[:])
                nc.vector.tensor_copy(qT[:, i * 128:(i + 1) * 128], p[:, :128])
            for i in range(NKC):
                kc_t = chunks.tile([128, KD], F32, tag="kc")
                nc.scalar.dma_start(kc_t[:], k_sh[i * 128:(i + 1) * 128, :])
                p = ps.tile([128, 512], F32, tag="t512")
                nc.tensor.transpose(p[:, :128], kc_t[:], id32[:])
                nc.vector.tensor_copy(kT[:, i * 128:(i + 1) * 128], p[:, :128])
            # values: needed only at the tail; own queue, cast to bf16
            v16 = []
            for i in range(NKC):
                vc = chunks.tile([128, VD], F32, tag="vc")
                nc.gpsimd.dma_start(vc[:], v_sh[i * 128:(i + 1) * 128, :])
                vb = vals.tile([128, VD], BF16, tag="v16", name=f"v16_{i}")
                nc.vector.tensor_copy(vb[:], vc[:])
                v16.append(vb)

            # ---- projections: packed (2x64 h, n) = [W | W]^T @ xT ----
            qp2 = io.tile([128, NQH], F32, tag="qp2")
            kp2 = io.tile([128, NK], F32, tag="kp2")
            p = ps.tile([128, 512], F32, tag="t512")
            nc.tensor.matmul(p[:], wq2_sb[:], qT[:], start=True, stop=True)
            nc.scalar.copy(qp2[:], p[:])
            for c in range(2):
                p = ps.tile([128, 512], F32, tag="t512")
                nc.tensor.matmul(p[:], wk2_sb[:], kT[:, c * 512:(c + 1) * 512],
                                 start=True, stop=True)
                nc.scalar.copy(kp2[:, c * 512:(c + 1) * 512], p[:])

            # ---- scoresT accumulation over j (transposed: k parts, q free) ----
            # Q side rows: [sin | cos], scaled by c_j*w_v; K side rows: [cos | sin]
            psT = [ps.tile([128, 512], F32, tag="t512", name=f"psT_{kt}")
                   for kt in range(NKC)]

            def emit_j(j, first, last):
                s1 = float(FOURIER_W[j] / TWO_PI)
                ks = jbank.tile([128, NK], BF16, tag="ks", name=f"ks_{j}")
                sq_f = work.tile([128, NQH], F32, tag="sqf", name=f"sqf_{j}")
                sq = jbank.tile([128, NQH], BF16, tag="sq", name=f"sq_{j}")
                if j == 0:  # |w0 x + phase| < 2.9: direct activation
                    nc.scalar.activation(ks[:], kp2[:], A.Sin,
                                         bias=biask_sb[:], scale=float(FOURIER_W[j]))
                    nc.scalar.activation(sq_f[:], qp2[:], A.Sin,
                                         bias=biasq_sb[:], scale=float(FOURIER_W[j]))
                else:
                    zk = work.tile([128, NK], F32, tag="zk", name=f"zk_{j}")
                    rk = work.tile([128, NK], F32, tag="rk", name=f"rk_{j}")
                    gk = work.tile([128, NK], F32, tag="gk", name=f"gk_{j}")
                    nc.vector.tensor_scalar(zk[:], kp2[:], s1, sphk_sb[:],
                                            AluOpType.mult, AluOpType.add)
                    nc.vector.tensor_scalar(rk[:], zk[:], MAGIC, MAGIC,
                                            AluOpType.add, AluOpType.subtract)
                    nc.gpsimd.tensor_sub(gk[:], rk[:], zk[:])
                    nc.scalar.activation(ks[:], gk[:], A.Sin, scale=-TWO_PI)
                    zq = work.tile([128, NQH], F32, tag="zq", name=f"zq_{j}")
                    rq = work.tile([128, NQH], F32, tag="rq", name=f"rq_{j}")
                    gq = work.tile([128, NQH], F32, tag="gq", name=f"gq_{j}")
                    nc.vector.tensor_scalar(zq[:], qp2[:], s1, sphq_sb[:],
                                            AluOpType.mult, AluOpType.add)
                    nc.vector.tensor_scalar(rq[:], zq[:], MAGIC, MAGIC,
                                            AluOpType.add, AluOpType.subtract)
                    nc.vector.tensor_sub(gq[:], rq[:], zq[:])
                    nc.scalar.activation(sq_f[:], gq[:], A.Sin, scale=-TWO_PI)
                # c_j*w_v scaling + bf16 cast on ScalarE (Copy with AP scale)
                nc.scalar.mul(sq[:], sq_f[:], cw_sb[:, j:j + 1])
                for kt in range(NKC):
                    nc.tensor.matmul(psT[kt][:],
                                     ks[:, kt * 128:(kt + 1) * 128], sq[:],
                                     start=first, stop=last)

            for idx, j in enumerate(J_ORDER):
                emit_j(j, first=(idx == 0), last=(idx == J - 1))

            # ---- exp (k-major) + denominators via ones-matmul ----
            expT = []
            for kt in range(NKC):
                et = sm.tile([128, 512], BF16, tag="expT", name=f"expT_{kt}")
                nc.scalar.activation(et[:], psT[kt][:], A.Exp)
                expT.append(et)
            psum_sums = ps.tile([1, 512], F32, tag="t512", name="psum_sums")
            for kt in range(NKC):
                nc.tensor.matmul(psum_sums[:], ones16[:], expT[kt][:],
                                 start=(kt == 0), stop=(kt == NKC - 1))
            sums_sb = sm.tile([1, 512], F32, tag="sums_sb")
            nc.scalar.copy(sums_sb[:], psum_sums[:])

            # ---- outT (v, q) = sum_kt values[kt] (as lhsT) @ expT[kt] ----
            ps_outT = ps.tile([128, 512], F32, tag="t512", name="ps_outT")
            for kt in range(NKC):
                nc.tensor.matmul(ps_outT[:], v16[kt][:], expT[kt][:],
                                 start=(kt == 0), stop=(kt == NKC - 1))
            outT_sb = sm.tile([128, 512], F32, tag="outT_sb")
            nc.vector.tensor_copy(outT_sb[:], ps_outT[:])

            # ---- transpose back to (q, v), normalize, store ----
            for qt in range(NQC):
                # recip of sums for this q tile as a per-partition column
                pcol = ps.tile([128, 512], F32, tag="t512", name=f"pcol_{qt}")
                # (1,128) row -> (128,1) column via 1-deep matmul against [[1.0]]
                nc.tensor.matmul(pcol[:128, :1],
                                 sums_sb[:1, qt * 128:(qt + 1) * 128],
                                 id32[:1, :1], start=True, stop=True)
                rcol = sm.tile([128, 1], F32, tag="rcol", name=f"rcol_{qt}")
                nc.vector.reciprocal(rcol[:], pcol[:128, :1])
                po = ps.tile([128, 512], F32, tag="t512", name=f"po_{qt}")
                nc.tensor.transpose(po[:, :128],
                                    outT_sb[:, qt * 128:(qt + 1) * 128], id32[:])
                o_sb = sm.tile([128, VD], F32, tag="osb", name=f"osb_{qt}")
                nc.vector.tensor_scalar_mul(o_sb[:], po[:, :128], rcol[:])
                nc.sync.dma_start(out[qt * 128:(qt + 1) * 128, :], o_sb[:])

    nc.finalize()
    return nc


def _get_nc():
    if "nc" not in _CACHED:
        _CACHED["nc"] = _build()
    return _CACHED["nc"]


def _make_consts(W_q, W_k, w_v):
    wq2 = np.concatenate([W_q, W_q], axis=1).astype(np.float32)
    wk2 = np.concatenate([W_k, W_k], axis=1).astype(np.float32)
    cw = np.zeros((128, J), np.float32)
    for j in range(J):
        cwj = (FOURIER_C[j] * w_v).astype(np.float32)
        cw[:64, j] = cwj
        cw[64:, j] = cwj
    sphq = np.full((128, 1), 32.0, np.float32)
    sphq[64:] = 32.25
    sphk = np.full((128, 1), 32.25, np.float32)
    sphk[64:] = 32.0
    biasq = np.zeros((128, 1), np.float32)
    biasq[64:] = HALF_PI
    biask = np.full((128, 1), HALF_PI, np.float32)
    biask[64:] = 0.0
    return wq2, wk2, cw, sphq, sphk, biasq, biask


def kernel(queries, keys, values, W_q, W_k, w_v, _trace=False, _trace_kwargs=None):
    from concourse.bass_utils import run_bass_kernel_spmd

    nc = _get_nc()
    wq2, wk2, cw, sphq, sphk, biasq, biask = _make_consts(
        np.asarray(W_q), np.asarray(W_k), np.asarray(w_v))
    queries = np.ascontiguousarray(queries, np.float32)
    keys = np.ascontiguousarray(keys, np.float32)
    values = np.ascontiguousarray(values, np.float32)

    in_maps = []
    for c in range(NCORES):
        b, qh = c // 2, c % 2
        in_maps.append({
            "q_sh": np.ascontiguousarray(queries[b, qh * NQH:(qh + 1) * NQH, :]),
            "k_sh": keys[b],
            "v_sh": values[b],
            "wq2": wq2, "wk2": wk2, "cw": cw, "sphq": sphq, "sphk": sphk,
            "biasq": biasq, "biask": biask,
        })

    kwargs = {}
    if _trace:
        kwargs["trace"] = True
        kwargs.update(_trace_kwargs or {})
    res = run_bass_kernel_spmd(nc, in_maps, core_ids=list(range(NCORES)), **kwargs)

    out = np.empty((BS, NQ, VD), np.float32)
    for c in range(NCORES):
        b, qh = c // 2, c % 2
        out[b, qh * NQH:(qh + 1) * NQH, :] = res.results[c]["out"]
    if _trace:
        return out, res
    return out
